# revision 27
# baseline (speedup 1.0000x reference)
"""Spatial self-attention scores kernel for Trainium2 (8 NeuronCores).

Computes, per batch b:
    qk = W @ x_b          # [256, 4096] = [256,256] @ [256,4096]
    q, k = qk[:128], qk[128:]
    sim = (q.T @ k) * 128**-0.5
    out_b = softmax(sim, axis=-1)        # [4096, 4096]
Output: [8, 1, 4096, 4096] float32.

Sharding: data-parallel over batch, one batch image per NeuronCore.

Per-core pipeline (all phases overlap under the Tile scheduler):
  - x DMA'd in as raw fp32 via HWDGE (no SWDGE cast DMA and its ~3 us
    Q7 descriptor-emission latency), four 1 MiB quarter transfers;
    DVE casts each quarter to fp16 as it arrives.
  - fp16 projection matmuls -> q,k cast to fp16 in SBUF [d=128,
    s=4096] on the PSUM->SBUF copy.
  - per 128-query row-tile: 8 fp16 matmuls (K=128, N=512) into 4-bank
    PSUM tiles; one ScalarE ACTIVATE per 2048 columns computes
    exp(SCALE*sim) with a fused row-sum (accum_out), writing fp16;
    DVE combines the partial sums, takes the reciprocal, and scales
    the row (4x perf mode on fp16).
  - ScalarE is the critical engine (~131 us of exp). The schedule
    keeps its PSUM ping-pong two chunks ahead: projection PSUM
    allocations are inserted in PAIRS (an odd insertion flips the
    2-buffer ring parity and costs an ACT bubble), and the first three
    groups run their first column chunk before any second chunk so k
    half-1's projection hides behind useful exps.
  - output rows leave as fp16 in 2 MB DMAs (two row-tiles per
    transfer; the last group ships per half-row to shorten the drain)
    and are upcast to fp32 on the host.
"""

import numpy as np
from contextlib import ExitStack

import concourse.bass as bass
import concourse.tile as tile
from concourse import bacc, mybir
from concourse.bass_utils import run_bass_kernel_spmd
from concourse.masks import make_identity

B = 8
C = 256
HW = 4096
D = 128
SCALE = D ** -0.5
N_CORES = 8

BANK = 512             # PSUM bank width (fp32) = one matmul free-dim
ACT_CHUNK = 2048       # one ScalarE activation spans 4 banks
N_ACT = HW // ACT_CHUNK          # 2
GRP = 2                # row-tiles per output DMA (2 -> 2 MB transfers)
N_GRP = HW // (128 * GRP)        # 16
OUT_BUFS = 6
X_CHUNK = 1024         # x input DMA granularity (overlaps with projection)

F32 = mybir.dt.float32
# The whole datapath runs in fp16: projection, q/k, the attention
# matmuls (PE 1 cycle/row, PSUM accumulates fp32), and the output.
# fp16's 10-bit mantissa keeps the end-to-end error ~6e-4
# scale-relative vs the 2e-2 gate, and halving the 64 MiB output
# write buys ~90 us/core.
PROJ_DT = mybir.dt.float16
OUT_DT = mybir.dt.float16


def _emit(ctx: ExitStack, tc: tile.TileContext, out_ap, x_ap, w_ap):
    nc = tc.nc

    const = ctx.enter_context(tc.tile_pool(name="const", bufs=1))
    data = ctx.enter_context(tc.tile_pool(name="data", bufs=1))
    psum = ctx.enter_context(tc.tile_pool(name="psum", bufs=2, space="PSUM"))
    small = ctx.enter_context(tc.tile_pool(name="small", bufs=8))

    # x lands as raw fp32 quarters via HWDGE; DVE casts each quarter
    # to fp16 at its scheduled point below (the DVE is in-order, so a
    # cast emitted too early would block every later DVE op behind
    # that quarter's DMA). Queue order: x quarter 0 first (it gates
    # the first exp chunk), then W, then the remaining quarters.
    x_view = x_ap.rearrange("(t p) s -> p t s", p=128)
    x0_f32 = data.tile([128, ACT_CHUNK // X_CHUNK, 2, X_CHUNK], F32)
    x1_f32 = data.tile([128, ACT_CHUNK // X_CHUNK, 2, X_CHUNK], F32)
    w_sb = const.tile([128, 2, C], F32)

    def load_x(half, c):
        dst_x = (x0_f32, x1_f32)[half]
        src = slice(half * ACT_CHUNK + c * X_CHUNK,
                    half * ACT_CHUNK + (c + 1) * X_CHUNK)
        nc.sync.dma_start(out=dst_x[:, c], in_=x_view[:, :, src])

    load_x(0, 0)
    nc.sync.dma_start(out=w_sb, in_=w_ap.rearrange("(t p) c -> p t c", p=128))
    load_x(0, 1)
    load_x(1, 0)
    load_x(1, 1)

    # ---- PE warm-up: throwaway matmuls while x is loading. The PE
    # clock gate (HAM) only releases to 2.4 GHz after ~3.4 us of
    # sustained activity; warming during the input DMA makes the
    # projection and the first attention row-tiles run at full rate.
    # A single DVE memset (~0.2 us, the DVE's first op) feeds the warm
    # tile; the results land in a throwaway PSUM tile, never read.
    warm = const.tile([128, BANK], PROJ_DT)
    nc.vector.memset(warm, 0.0)
    wps = psum.tile([128, ACT_CHUNK], F32, tag="ps")
    for _ in range(6):
        nc.tensor.matmul(
            wps[:, 0:BANK], warm[:, 0:128], warm, start=True, stop=True
        )
    warm_f32 = const.tile([128, BANK], F32)
    nc.vector.memset(warm_f32, 0.0)

    ident = const.tile([128, 128], F32)
    make_identity(nc, ident)

    # pull the exp table load off the first real activation
    tbl = small.tile([128, 1], F32, tag="tbl")
    nc.scalar.activation(
        out=tbl, in_=warm_f32[:, 0:1], func=mybir.ActivationFunctionType.Exp
    )

    # fp16 x staging; cast_x(half, c) is invoked at schedule points
    x0_sb = data.tile([128, ACT_CHUNK // X_CHUNK, 2, X_CHUNK], PROJ_DT)
    x1_sb = data.tile([128, ACT_CHUNK // X_CHUNK, 2, X_CHUNK], PROJ_DT)

    def cast_x(half, c):
        src_x = (x0_f32, x1_f32)[half]
        dst_x = (x0_sb, x1_sb)[half]
        nc.vector.tensor_copy(out=dst_x[:, c], in_=src_x[:, c])

    cast_x(0, 0)

    # ---- transpose W on PE -> wt_sb[c_sub, c_tile, o] (contraction c on partitions)
    wt_sb = const.tile([128, 2, 2 * D], PROJ_DT)
    for t in range(2):          # output-channel tile (q half / k half)
        for ct in range(2):     # input-channel tile
            ps = psum.tile([128, ACT_CHUNK], F32, tag="ps")
            nc.tensor.transpose(
                ps[:, 0:128], w_sb[:, t, ct * 128:(ct + 1) * 128], ident
            )
            nc.vector.tensor_copy(
                out=wt_sb[:, ct, t * 128:(t + 1) * 128], in_=ps[:, 0:128]
            )

    q_sb = data.tile([128, HW], PROJ_DT)
    k_sb = data.tile([128, HW], PROJ_DT)

    def proj_chunk(t, dst, a, x_half, banks=None):
        """Project output-channel half t for column chunk a; x_half is
        [128, chunk, c_tile, X_CHUNK] holding x columns
        [a*ACT_CHUNK, (a+1)*ACT_CHUNK). banks selects a contiguous
        subset of the four 512-wide banks. Banks sharing an x chunk
        run as one wide matmul pair (fewer LDWEIGHTS); one PSUM
        allocation and one PSUM->SBUF fp16 cast per call."""
        if banks is None:
            banks = tuple(range(ACT_CHUNK // BANK))
        ps = psum.tile([128, ACT_CHUNK], F32, tag="ps")
        for jj in banks:
            lo = slice(jj * BANK, (jj + 1) * BANK)
            ch = (jj * BANK) // X_CHUNK
            off = (jj * BANK) % X_CHUNK
            nc.tensor.matmul(
                ps[:, lo], wt_sb[:, 0, t * 128:(t + 1) * 128],
                x_half[:, ch, 0, off:off + BANK], start=True, stop=False,
            )
            nc.tensor.matmul(
                ps[:, lo], wt_sb[:, 1, t * 128:(t + 1) * 128],
                x_half[:, ch, 1, off:off + BANK], start=False, stop=True,
            )
        lo_c = a * ACT_CHUNK + banks[0] * BANK
        hi_c = a * ACT_CHUNK + (banks[-1] + 1) * BANK
        nc.vector.tensor_copy(
            out=dst[:, lo_c:hi_c],
            in_=ps[:, banks[0] * BANK:(banks[-1] + 1) * BANK],
        )

    out_view = out_ap.rearrange("(g t p) m -> g p t m", t=GRP, p=128)

    def sim_chunk(lhs, out_row, lo_col, n_col, accum):
        """n_col-wide slice of one attention row: per-bank matmuls
        (the ISA caps a matmul's free dim at one 512-wide PSUM bank)
        + fused exp."""
        ps = psum.tile([128, ACT_CHUNK], F32, tag="ps")
        for jj in range(n_col // BANK):
            sl = slice(lo_col + jj * BANK, lo_col + (jj + 1) * BANK)
            nc.tensor.matmul(
                ps[:, jj * BANK:(jj + 1) * BANK], lhs, k_sb[:, sl],
                start=True, stop=True,
            )
        nc.scalar.activation(
            out=out_row[:, lo_col:lo_col + n_col],
            in_=ps[:, 0:n_col],
            func=mybir.ActivationFunctionType.Exp,
            scale=SCALE,
            accum_out=accum,
        )

    pend = {}

    def open_group(g, fine=False):
        """Emit column chunk 0 of both row-tiles of group g. fine=True
        splits row 0's chunk 0 into 1024-wide pieces chasing the x
        quarter DMAs."""
        out_grp = outp.tile([128, GRP, HW], OUT_DT, tag="out")
        sums = [small.tile([128, N_ACT + 1], F32, tag="sums", name="sums")
                for _ in range(GRP)]
        pend[g] = (out_grp, sums)
        for t in range(GRP):
            i = g * GRP + t
            lhs = q_sb[:, i * 128:(i + 1) * 128]
            if fine and t == 0:
                sim_chunk(lhs, out_grp[:, t], 0, 1024, sums[t][:, 0:1])
                cast_x(0, 1)                      # x cols 1024:2048
                proj_chunk(1, k_sb, 0, x0_sb, banks=(2, 3))
                sim_chunk(lhs, out_grp[:, t], 1024, 1024, sums[t][:, 1:2])
                cast_x(1, 0)                      # x cols 2048:3072
                proj_chunk(1, k_sb, 1, x1_sb, banks=(0, 1))
            else:
                sim_chunk(lhs, out_grp[:, t], 0, ACT_CHUNK, sums[t][:, 0:1])

    def close_group(g, split_dma=False):
        """Emit the last chunk of both row-tiles, normalize, ship."""
        out_grp, sums = pend.pop(g)
        for t in range(GRP):
            i = g * GRP + t
            slot = 2 if (g == 0 and t == 0) else 1
            sim_chunk(q_sb[:, i * 128:(i + 1) * 128], out_grp[:, t],
                      ACT_CHUNK, ACT_CHUNK, sums[t][:, slot:slot + 1])
            rsum = small.tile([128, 1], F32, tag="rsum")
            nc.vector.tensor_reduce(
                out=rsum, in_=sums[t][:, 0:slot + 1],
                axis=mybir.AxisListType.X, op=mybir.AluOpType.add,
            )
            recip = small.tile([128, 1], F32, tag="recip")
            nc.vector.reciprocal(out=recip, in_=rsum)
            if split_dma:
                # normalize and ship each half-row as soon as it is
                # scaled (1 MB transfers) to shorten the final drain
                for a in range(N_ACT):
                    sl = slice(a * ACT_CHUNK, (a + 1) * ACT_CHUNK)
                    nc.vector.tensor_scalar_mul(
                        out=out_grp[:, t, sl], in0=out_grp[:, t, sl],
                        scalar1=recip,
                    )
                    nc.sync.dma_start(
                        out=out_ap[i * 128:(i + 1) * 128, sl],
                        in_=out_grp[:, t, sl],
                    )
            else:
                nc.vector.tensor_scalar_mul(
                    out=out_grp[:, t, :], in0=out_grp[:, t, :], scalar1=recip
                )
        if not split_dma:
            nc.sync.dma_start(out=out_view[g], in_=out_grp)

    # ---- schedule. Projection pieces are placed as early as their x
    # quarters allow, and (once the exp stream is dense) only as
    # ADJACENT PAIRS of PSUM allocations: the sim/ACT ping-pong runs
    # two chunks ahead, and an odd insertion flips the 2-buffer ring
    # parity, costing a ~1.8 us ACT bubble.
    proj_chunk(1, k_sb, 0, x0_sb, banks=(0, 1))   # k cols    0:1024  [x q0]
    proj_chunk(0, q_sb, 0, x0_sb, banks=(0,))     # q rows    0:512   [x q0]

    outp = ctx.enter_context(tc.tile_pool(name="outp", bufs=OUT_BUFS))
    open_group(0, fine=True)          # r0 in 1024-pieces + k interleave, r1c0
    cast_x(1, 1)                                  # x cols 3072:4096
    proj_chunk(1, k_sb, 1, x1_sb, banks=(2, 3))   # k cols 3072:4096  [x q3]
    proj_chunk(0, q_sb, 0, x0_sb, banks=(1,))     # q rows  512:1024
    open_group(1)                                 # r2c0, r3c0
    close_group(0)                                # r0c2, r1c1
    open_group(2)                                 # r4c0, r5c0
    close_group(1)                                # r2c1, r3c1
    proj_chunk(0, q_sb, 0, x0_sb, banks=(2,))     # q rows 1024:1536
    proj_chunk(0, q_sb, 0, x0_sb, banks=(3,))     # q rows 1536:2048
    open_group(3)                                 # r6c0, r7c0
    close_group(2)                                # r4c1, r5c1
    proj_chunk(0, q_sb, 1, x1_sb, banks=(0, 1))   # q rows 2048:3072
    proj_chunk(0, q_sb, 1, x1_sb, banks=(2, 3))   # q rows 3072:4096
    for g in range(4, N_GRP):
        open_group(g)
        close_group(g - 1)
    close_group(N_GRP - 1, split_dma=True)


_built = None


def _get_nc():
    global _built
    if _built is None:
        nc = bacc.Bacc("TRN2", target_bir_lowering=False, debug=False)
        x = nc.dram_tensor("x", [C, HW], F32, kind="ExternalInput").ap()
        w = nc.dram_tensor("w", [2 * D, C], F32, kind="ExternalInput").ap()
        out = nc.dram_tensor("out", [HW, HW], OUT_DT, kind="ExternalOutput").ap()
        with tile.TileContext(nc) as tc:
            with ExitStack() as ctx:
                _emit(ctx, tc, out, x, w)
        nc.compile()
        _built = nc
    return _built


def kernel(x: np.ndarray, W: np.ndarray) -> np.ndarray:
    nc = _get_nc()
    x = np.asarray(x, dtype=np.float32)
    W = np.ascontiguousarray(np.asarray(W, dtype=np.float32))
    in_maps = [
        {"x": np.ascontiguousarray(x[b].reshape(C, HW)), "w": W} for b in range(B)
    ]
    res = run_bass_kernel_spmd(nc, in_maps, core_ids=list(range(N_CORES)))
    out = np.stack([res.results[b]["out"] for b in range(B)]).astype(np.float32)
    return out[:, None]


# revision 34
# speedup vs baseline: 1.0148x; 1.0148x over previous
"""Spatial self-attention scores kernel for Trainium2 (8 NeuronCores).

Computes, per batch b:
    qk = W @ x_b          # [256, 4096] = [256,256] @ [256,4096]
    q, k = qk[:128], qk[128:]
    sim = (q.T @ k) * 128**-0.5
    out_b = softmax(sim, axis=-1)        # [4096, 4096]
Output: [8, 1, 4096, 4096] float32.

Sharding: data-parallel over batch, one batch image per NeuronCore.

Per-core pipeline (all phases overlap under the Tile scheduler):
  - x DMA'd in as raw fp32 via HWDGE (no SWDGE cast DMA and its ~3 us
    Q7 descriptor-emission latency), four 1 MiB quarter transfers;
    DVE casts each quarter to fp16 as it arrives.
  - fp16 projection matmuls -> q,k cast to fp16 in SBUF [d=128,
    s=4096] on the PSUM->SBUF copy.
  - per 128-query row-tile: 8 fp16 matmuls (K=128, N=512) into 4-bank
    PSUM tiles; one ScalarE ACTIVATE per 2048 columns computes
    exp(SCALE*sim) with a fused row-sum (accum_out), writing fp16;
    DVE combines the partial sums, takes the reciprocal, and scales
    the row (4x perf mode on fp16).
  - ScalarE is the critical engine (~131 us of exp). The schedule
    keeps its PSUM ping-pong two chunks ahead: projection PSUM
    allocations are inserted in PAIRS (an odd insertion flips the
    2-buffer ring parity and costs an ACT bubble), and the first three
    groups run their first column chunk before any second chunk so k
    half-1's projection hides behind useful exps.
  - output rows leave as fp16 in 2 MB DMAs (two row-tiles per
    transfer; the last group ships per half-row to shorten the drain)
    and are upcast to fp32 on the host.
"""

import numpy as np
from contextlib import ExitStack

import concourse.bass as bass
import concourse.tile as tile
from concourse import bacc, mybir
from concourse.bass_utils import run_bass_kernel_spmd
from concourse.masks import make_identity

B = 8
C = 256
HW = 4096
D = 128
SCALE = D ** -0.5
N_CORES = 8

BANK = 512             # PSUM bank width (fp32) = one matmul free-dim
ACT_CHUNK = 2048       # one ScalarE activation spans 4 banks
N_ACT = HW // ACT_CHUNK          # 2
GRP = 2                # row-tiles per output DMA (2 -> 2 MB transfers)
N_GRP = HW // (128 * GRP)        # 16
OUT_BUFS = 6
X_CHUNK = 1024         # x input DMA granularity (overlaps with projection)

F32 = mybir.dt.float32
# The whole datapath runs in fp16: projection, q/k, the attention
# matmuls (PE 1 cycle/row, PSUM accumulates fp32), and the output.
# fp16's 10-bit mantissa keeps the end-to-end error ~6e-4
# scale-relative vs the 2e-2 gate, and halving the 64 MiB output
# write buys ~90 us/core.
PROJ_DT = mybir.dt.float16
OUT_DT = mybir.dt.float16


def _emit(ctx: ExitStack, tc: tile.TileContext, out_ap, x_ap, w_ap):
    nc = tc.nc

    const = ctx.enter_context(tc.tile_pool(name="const", bufs=1))
    data = ctx.enter_context(tc.tile_pool(name="data", bufs=1))
    psum = ctx.enter_context(tc.tile_pool(name="psum", bufs=2, space="PSUM"))
    small = ctx.enter_context(tc.tile_pool(name="small", bufs=8))

    # x lands as raw fp32 quarters via HWDGE; DVE casts each quarter
    # to fp16 at its scheduled point below (the DVE is in-order, so a
    # cast emitted too early would block every later DVE op behind
    # that quarter's DMA). Queue order: x quarter 0 first (it gates
    # the first exp chunk), then W, then the remaining quarters.
    x_view = x_ap.rearrange("(t p) s -> p t s", p=128)
    x0_f32 = data.tile([128, ACT_CHUNK // X_CHUNK, 2, X_CHUNK], F32)
    x1_f32 = data.tile([128, ACT_CHUNK // X_CHUNK, 2, X_CHUNK], F32)
    w_sb = const.tile([128, 2, C], F32)

    def load_x(half, c):
        dst_x = (x0_f32, x1_f32)[half]
        src = slice(half * ACT_CHUNK + c * X_CHUNK,
                    half * ACT_CHUNK + (c + 1) * X_CHUNK)
        nc.sync.dma_start(out=dst_x[:, c], in_=x_view[:, :, src])

    # W first (it gates the weight transposes, which gate ALL
    # projection); x quarters 1-3 behind it on the same FIFO ring.
    nc.sync.dma_start(out=w_sb, in_=w_ap.rearrange("(t p) c -> p t c", p=128))
    load_x(0, 1)
    load_x(1, 0)
    load_x(1, 1)

    # ---- PE warm-up: throwaway matmuls while x is loading. The PE
    # clock gate (HAM) only releases to 2.4 GHz after ~3.4 us of
    # sustained activity; warming during the input DMA makes the
    # projection and the first attention row-tiles run at full rate.
    # A single DVE memset (~0.2 us, the DVE's first op) feeds the warm
    # tile; the results land in a throwaway PSUM tile, never read.
    # Enough reps to keep the PE clocking until the projection starts
    # (~13 us) -- the HAM gate re-engages after ~1 us of PE idle.
    warm = const.tile([128, BANK], PROJ_DT)
    nc.vector.memset(warm, 0.0)
    wps = psum.tile([128, ACT_CHUNK], F32, tag="ps")
    for _ in range(12):
        nc.tensor.matmul(
            wps[:, 0:BANK], warm[:, 0:128], warm, start=True, stop=True
        )
    warm_f32 = const.tile([128, BANK], F32)
    nc.vector.memset(warm_f32, 0.0)

    ident = const.tile([128, 128], F32)
    make_identity(nc, ident)

    # pull the exp table load off the first real activation
    tbl = small.tile([128, 1], F32, tag="tbl")
    nc.scalar.activation(
        out=tbl, in_=warm_f32[:, 0:1], func=mybir.ActivationFunctionType.Exp
    )

    # fp16 x staging; cast_x(half, c) is invoked at schedule points.
    # Quarter 0 skips the fp32 staging entirely: an SWDGE cast DMA on
    # the gpsimd queue writes it straight to fp16, concurrently with
    # the HWDGE stream above (separate engine + ring), and saves the
    # in-chain DVE cast that gates the very first exp chunk.
    x0_sb = data.tile([128, ACT_CHUNK // X_CHUNK, 2, X_CHUNK], PROJ_DT)
    x1_sb = data.tile([128, ACT_CHUNK // X_CHUNK, 2, X_CHUNK], PROJ_DT)
    nc.gpsimd.dma_start(out=x0_sb[:, 0], in_=x_view[:, :, 0:X_CHUNK])

    def cast_x(half, c):
        src_x = (x0_f32, x1_f32)[half]
        dst_x = (x0_sb, x1_sb)[half]
        nc.vector.tensor_copy(out=dst_x[:, c], in_=src_x[:, c])

    # ---- transpose W on PE -> wt_sb[c_sub, c_tile, o] (contraction c on partitions)
    wt_sb = const.tile([128, 2, 2 * D], PROJ_DT)
    for t in range(2):          # output-channel tile (q half / k half)
        for ct in range(2):     # input-channel tile
            ps = psum.tile([128, ACT_CHUNK], F32, tag="ps")
            nc.tensor.transpose(
                ps[:, 0:128], w_sb[:, t, ct * 128:(ct + 1) * 128], ident
            )
            nc.vector.tensor_copy(
                out=wt_sb[:, ct, t * 128:(t + 1) * 128], in_=ps[:, 0:128]
            )

    cast_x(0, 1)                                  # x cols 1024:2048

    q_sb = data.tile([128, HW], PROJ_DT)
    k_sb = data.tile([128, HW], PROJ_DT)

    def proj_chunk(t, dst, a, x_half, banks=None):
        """Project output-channel half t for column chunk a; x_half is
        [128, chunk, c_tile, X_CHUNK] holding x columns
        [a*ACT_CHUNK, (a+1)*ACT_CHUNK). banks selects a contiguous
        subset of the four 512-wide banks. Banks sharing an x chunk
        run as one wide matmul pair (fewer LDWEIGHTS); one PSUM
        allocation and one PSUM->SBUF fp16 cast per call."""
        if banks is None:
            banks = tuple(range(ACT_CHUNK // BANK))
        ps = psum.tile([128, ACT_CHUNK], F32, tag="ps")
        for jj in banks:
            lo = slice(jj * BANK, (jj + 1) * BANK)
            ch = (jj * BANK) // X_CHUNK
            off = (jj * BANK) % X_CHUNK
            nc.tensor.matmul(
                ps[:, lo], wt_sb[:, 0, t * 128:(t + 1) * 128],
                x_half[:, ch, 0, off:off + BANK], start=True, stop=False,
            )
            nc.tensor.matmul(
                ps[:, lo], wt_sb[:, 1, t * 128:(t + 1) * 128],
                x_half[:, ch, 1, off:off + BANK], start=False, stop=True,
            )
        lo_c = a * ACT_CHUNK + banks[0] * BANK
        hi_c = a * ACT_CHUNK + (banks[-1] + 1) * BANK
        nc.vector.tensor_copy(
            out=dst[:, lo_c:hi_c],
            in_=ps[:, banks[0] * BANK:(banks[-1] + 1) * BANK],
        )

    out_view = out_ap.rearrange("(g t p) m -> g p t m", t=GRP, p=128)

    def sim_chunk(lhs, out_row, lo_col, n_col, accum):
        """n_col-wide slice of one attention row: per-bank matmuls
        (the ISA caps a matmul's free dim at one 512-wide PSUM bank)
        + fused exp."""
        ps = psum.tile([128, ACT_CHUNK], F32, tag="ps")
        for jj in range(n_col // BANK):
            sl = slice(lo_col + jj * BANK, lo_col + (jj + 1) * BANK)
            nc.tensor.matmul(
                ps[:, jj * BANK:(jj + 1) * BANK], lhs, k_sb[:, sl],
                start=True, stop=True,
            )
        nc.scalar.activation(
            out=out_row[:, lo_col:lo_col + n_col],
            in_=ps[:, 0:n_col],
            func=mybir.ActivationFunctionType.Exp,
            scale=SCALE,
            accum_out=accum,
        )

    pend = {}

    def open_group(g, fine=False):
        """Emit column chunk 0 of both row-tiles of group g. fine=True
        splits row 0's chunk 0 into 1024-wide pieces chasing the x
        quarter DMAs."""
        out_grp = outp.tile([128, GRP, HW], OUT_DT, tag="out")
        sums = [small.tile([128, N_ACT + 1], F32, tag="sums", name="sums")
                for _ in range(GRP)]
        pend[g] = (out_grp, sums)
        for t in range(GRP):
            i = g * GRP + t
            lhs = q_sb[:, i * 128:(i + 1) * 128]
            if fine and t == 0:
                sim_chunk(lhs, out_grp[:, t], 0, 1024, sums[t][:, 0:1])
                proj_chunk(1, k_sb, 0, x0_sb, banks=(2, 3))
                proj_chunk(0, q_sb, 0, x0_sb, banks=(1,))
                sim_chunk(lhs, out_grp[:, t], 1024, 1024, sums[t][:, 1:2])
            else:
                sim_chunk(lhs, out_grp[:, t], 0, ACT_CHUNK, sums[t][:, 0:1])

    def close_group(g, split_dma=False):
        """Emit the last chunk of both row-tiles, normalize, ship."""
        out_grp, sums = pend.pop(g)
        for t in range(GRP):
            i = g * GRP + t
            slot = 2 if (g == 0 and t == 0) else 1
            sim_chunk(q_sb[:, i * 128:(i + 1) * 128], out_grp[:, t],
                      ACT_CHUNK, ACT_CHUNK, sums[t][:, slot:slot + 1])
            rsum = small.tile([128, 1], F32, tag="rsum")
            nc.vector.tensor_reduce(
                out=rsum, in_=sums[t][:, 0:slot + 1],
                axis=mybir.AxisListType.X, op=mybir.AluOpType.add,
            )
            recip = small.tile([128, 1], F32, tag="recip")
            nc.vector.reciprocal(out=recip, in_=rsum)
            if split_dma:
                # normalize and ship each half-row as soon as it is
                # scaled (1 MB transfers) to shorten the final drain
                for a in range(N_ACT):
                    sl = slice(a * ACT_CHUNK, (a + 1) * ACT_CHUNK)
                    nc.vector.tensor_scalar_mul(
                        out=out_grp[:, t, sl], in0=out_grp[:, t, sl],
                        scalar1=recip,
                    )
                    nc.sync.dma_start(
                        out=out_ap[i * 128:(i + 1) * 128, sl],
                        in_=out_grp[:, t, sl],
                    )
            else:
                nc.vector.tensor_scalar_mul(
                    out=out_grp[:, t, :], in0=out_grp[:, t, :], scalar1=recip
                )
        if not split_dma:
            nc.sync.dma_start(out=out_view[g], in_=out_grp)

    # ---- schedule. Projection pieces are placed as early as their x
    # quarters allow, and (once the exp stream is dense) only as
    # ADJACENT PAIRS of PSUM allocations: the sim/ACT ping-pong runs
    # two chunks ahead, and an odd insertion flips the 2-buffer ring
    # parity, costing a ~1.8 us ACT bubble.
    proj_chunk(1, k_sb, 0, x0_sb, banks=(0, 1))   # k cols    0:1024  [x q0]
    proj_chunk(0, q_sb, 0, x0_sb, banks=(0,))     # q rows    0:512   [x q0]

    outp = ctx.enter_context(tc.tile_pool(name="outp", bufs=OUT_BUFS))
    open_group(0, fine=True)        # r0 in 1024-pieces + k b23/q b1, r1c0
    cast_x(1, 0)                                  # x cols 2048:3072
    proj_chunk(1, k_sb, 1, x1_sb, banks=(0, 1))   # k cols 2048:3072  [x q2]
    proj_chunk(0, q_sb, 0, x0_sb, banks=(2,))     # q rows 1024:1536
    open_group(1)                                 # r2c0, r3c0
    cast_x(1, 1)                                  # x cols 3072:4096
    proj_chunk(1, k_sb, 1, x1_sb, banks=(2, 3))   # k cols 3072:4096  [x q3]
    proj_chunk(0, q_sb, 0, x0_sb, banks=(3,))     # q rows 1536:2048
    open_group(2)                                 # r4c0, r5c0
    close_group(0)                                # r0c2, r1c1
    open_group(3)                                 # r6c0, r7c0
    close_group(1)                                # r2c1, r3c1
    proj_chunk(0, q_sb, 1, x1_sb, banks=(0, 1))   # q rows 2048:3072
    proj_chunk(0, q_sb, 1, x1_sb, banks=(2, 3))   # q rows 3072:4096
    for g in range(4, N_GRP):
        open_group(g)
        close_group(g - 2)
    close_group(N_GRP - 2)
    close_group(N_GRP - 1, split_dma=True)


_built = None


def _get_nc():
    global _built
    if _built is None:
        nc = bacc.Bacc("TRN2", target_bir_lowering=False, debug=False)
        x = nc.dram_tensor("x", [C, HW], F32, kind="ExternalInput").ap()
        w = nc.dram_tensor("w", [2 * D, C], F32, kind="ExternalInput").ap()
        out = nc.dram_tensor("out", [HW, HW], OUT_DT, kind="ExternalOutput").ap()
        with tile.TileContext(nc) as tc:
            with ExitStack() as ctx:
                _emit(ctx, tc, out, x, w)
        nc.compile()
        _built = nc
    return _built


def kernel(x: np.ndarray, W: np.ndarray) -> np.ndarray:
    nc = _get_nc()
    x = np.asarray(x, dtype=np.float32)
    W = np.ascontiguousarray(np.asarray(W, dtype=np.float32))
    in_maps = [
        {"x": np.ascontiguousarray(x[b].reshape(C, HW)), "w": W} for b in range(B)
    ]
    res = run_bass_kernel_spmd(nc, in_maps, core_ids=list(range(N_CORES)))
    out = np.stack([res.results[b]["out"] for b in range(B)]).astype(np.float32)
    return out[:, None]
